# revision 8
# baseline (speedup 1.0000x reference)
import sys
sys.path.insert(0, "/opt/trn_rl_repo")
import numpy as np
import ml_dtypes
import concourse.mybir as mybir
import concourse.bacc as bacc
from concourse.bass import ds
from concourse.tile import TileContext
from concourse.bass_utils import run_bass_kernel_spmd
from concourse.masks import make_identity

F32 = mybir.dt.float32
BF16 = mybir.dt.bfloat16
BF = ml_dtypes.bfloat16
N_CORES = 8
NB = 4          # samples per core
C = 96
MID = 512
H = W = 32
HW = 1024
PW = 34
PHW = PW * PW   # 1156
EPS = 1e-5
AX = mybir.AxisListType.X
ADD = mybir.AluOpType.add
RELU = mybir.ActivationFunctionType.Relu
COPY = mybir.ActivationFunctionType.Copy
IDENT = mybir.ActivationFunctionType.Identity

_cache = {}


def _conv3x3(nc, pool, psum, lhsT_tiles, src_pad, dst_write, cout_blocks, n_cin, ps_bufs=6):
    """3x3 conv: src_pad tiles [P,NB,34,34]; lhsT_tiles[ci][:, t, co_sl] bf16.
    dst_write(co_blk, b, half, ps) consumes PSUM [P_out, 16, 32]."""
    for co in range(cout_blocks):
        for b in range(NB):
            for half in range(2):
                pout = 96 if cout_blocks == 1 else 128
                ps = psum.tile([pout, 16, 32], F32, tag="convps", name="convps", bufs=ps_bufs)
                first = True
                for ci in range(n_cin):
                    for t in range(9):
                        ky, kx = t // 3, t % 3
                        rhs = src_pad[ci][:, b, ds(16 * half + ky, 16), ds(kx, 32)]
                        co_sl = ds(co * 128, 128) if cout_blocks > 1 else ds(0, 96)
                        nc.tensor.matmul(
                            ps[:], lhsT_tiles[ci][:, t, co_sl], rhs,
                            start=first, stop=(ci == n_cin - 1 and t == 8),
                        )
                        first = False
                dst_write(co, b, half, ps)


def _stats(nc, pool, ps, sumb, sqb, idx, scratch):
    psf = ps[:].rearrange("p a b -> p (a b)")
    scf = scratch[:].rearrange("p a b -> p (a b)")
    nc.vector.tensor_reduce(sumb[:, ds(idx, 1)], psf, AX, ADD)
    nc.scalar.square(scf, psf)
    nc.vector.tensor_reduce(sqb[:, ds(idx, 1)], scf, AX, ADD)


def _build_l1():
    nc = bacc.Bacc("TRN2", target_bir_lowering=False, debug=False, num_devices=N_CORES)
    xc_d = nc.dram_tensor("xc", [NB, C, HW], F32, kind="ExternalInput").ap()
    s1_d = nc.dram_tensor("s1", [C, 2], F32, kind="ExternalInput").ap()
    w1_d = nc.dram_tensor("w1", [C, 9 * MID], BF16, kind="ExternalInput").ap()
    w2_d = nc.dram_tensor("w2", [MID, 9 * MID], BF16, kind="ExternalInput").ap()
    h2_d = nc.dram_tensor("h2", [4, 128, NB * HW], BF16, kind="ExternalOutput").ap()
    st_d = nc.dram_tensor("st1", [4, 128, 8, 2], F32, kind="ExternalOutput").ap()

    with TileContext(nc) as tc:
        with (
            tc.tile_pool(name="consts", bufs=1) as consts,
            tc.tile_pool(name="act", bufs=1) as act,
            tc.tile_pool(name="ps", bufs=2, space="PSUM") as psum,
            tc.tile_pool(name="tmp", bufs=4) as tmp,
        ):
            # weights
            w1 = consts.tile([C, 9, MID], BF16)
            nc.gpsimd.dma_start(out=w1[:], in_=w1_d.rearrange("c (t o) -> c t o", t=9))
            w2 = [consts.tile([128, 9, MID], BF16, tag=f"w2_{i}", name=f"w2_{i}") for i in range(4)]
            for i in range(4):
                nc.gpsimd.dma_start(
                    out=w2[i][:],
                    in_=w2_d[ds(128 * i, 128)].rearrange("c (t o) -> c t o", t=9),
                )
            s1 = consts.tile([C, 2], F32)
            nc.gpsimd.dma_start(out=s1[:], in_=s1_d)
            # x_cond -> bn1 -> padded bf16
            xc = act.tile([C, NB, HW], F32)
            nc.gpsimd.dma_start(out=xc[:], in_=xc_d.rearrange("b c n -> c b n"))
            xcp = act.tile([C, NB, PW, PW], BF16)
            nc.vector.memset(xcp[:], 0.0)
            for b in range(NB):
                nc.scalar.activation(
                    xcp[:, b, ds(1, 32), ds(1, 32)],
                    xc[:, b].rearrange("p (h w) -> p h w", h=32),
                    IDENT, bias=s1[:, ds(1, 1)], scale=s1[:, ds(0, 1)],
                )
            # conv_in + relu -> h1p
            h1p = [act.tile([128, NB, PW, PW], BF16, tag=f"h1_{i}", name=f"h1_{i}") for i in range(4)]
            for i in range(4):
                nc.vector.memset(h1p[i][:], 0.0)

            def wr1(co, b, half, ps):
                nc.scalar.activation(
                    h1p[co][:, b, ds(1 + 16 * half, 16), ds(1, 32)], ps[:], RELU
                )
            _conv3x3(nc, act, psum, [w1], [xcp], wr1, 4, 1)

            # conv_mid1 -> h2 + stats
            sumb = [act.tile([128, 8], F32, tag=f"su{i}", name=f"su{i}") for i in range(4)]
            sqb = [act.tile([128, 8], F32, tag=f"sq{i}", name=f"sq{i}") for i in range(4)]
            h2 = [act.tile([128, NB, HW], BF16, tag=f"h2_{i}", name=f"h2_{i}") for i in range(4)]
            scratch = tmp.tile([128, 16, 32], F32, tag="scr")

            def wr2(co, b, half, ps):
                sc = tmp.tile([128, 16, 32], F32, tag="scr")
                _stats(nc, act, ps, sumb[co], sqb[co], b * 2 + half, sc)
                nc.scalar.activation(
                    h2[co][:, b, ds(512 * half, 512)],
                    ps[:].rearrange("p a b -> p (a b)"), COPY,
                )
            _conv3x3(nc, act, psum, w2, h1p, wr2, 4, 4)

            for i in range(4):
                nc.gpsimd.dma_start(out=h2_d[i], in_=h2[i][:].rearrange("p b n -> p (b n)"))
                nc.gpsimd.dma_start(out=st_d[i, :, :, 0], in_=sumb[i][:])
                nc.gpsimd.dma_start(out=st_d[i, :, :, 1], in_=sqb[i][:])
    nc.compile()
    return nc


def _build_l2():
    nc = bacc.Bacc("TRN2", target_bir_lowering=False, debug=False, num_devices=N_CORES)
    h2_d = nc.dram_tensor("h2", [4, 128, NB * HW], BF16, kind="ExternalInput").ap()
    s2_d = nc.dram_tensor("s2", [4, 128, 2], F32, kind="ExternalInput").ap()
    w3_d = nc.dram_tensor("w3", [MID, MID], BF16, kind="ExternalInput").ap()
    h3_d = nc.dram_tensor("h3", [4, 128, NB * HW], BF16, kind="ExternalOutput").ap()
    st_d = nc.dram_tensor("st2", [4, 128, 8, 2], F32, kind="ExternalOutput").ap()

    with TileContext(nc) as tc:
        with (
            tc.tile_pool(name="consts", bufs=1) as consts,
            tc.tile_pool(name="act", bufs=1) as act,
            tc.tile_pool(name="ps", bufs=2, space="PSUM") as psum,
            tc.tile_pool(name="tmp", bufs=4) as tmp,
        ):
            w3 = [consts.tile([128, MID], BF16, tag=f"w3_{i}", name=f"w3_{i}") for i in range(4)]
            for i in range(4):
                nc.gpsimd.dma_start(out=w3[i][:], in_=w3_d[ds(128 * i, 128)])
            s2 = consts.tile([128, 4, 2], F32)
            nc.gpsimd.dma_start(out=s2[:], in_=s2_d.rearrange("k p c -> p k c"))
            h2b = [act.tile([128, NB, HW], BF16, tag=f"h2b{i}", name=f"h2b{i}") for i in range(4)]
            for i in range(4):
                hin = act.tile([128, NB * HW], BF16, tag=f"hin{i}")
                nc.gpsimd.dma_start(out=hin[:], in_=h2_d[i])
                nc.scalar.activation(
                    h2b[i][:].rearrange("p b n -> p (b n)"), hin[:],
                    RELU, bias=s2[:, i, ds(1, 1)], scale=s2[:, i, ds(0, 1)],
                )
            sumb = [act.tile([128, 8], F32, tag=f"su{i}", name=f"su{i}") for i in range(4)]
            sqb = [act.tile([128, 8], F32, tag=f"sq{i}", name=f"sq{i}") for i in range(4)]
            h3 = [act.tile([128, NB, HW], BF16, tag=f"h3_{i}", name=f"h3_{i}") for i in range(4)]
            for co in range(4):
                for b in range(NB):
                    for half in range(2):
                        ps = psum.tile([128, 512], F32, tag="ps", name="ps", bufs=6)
                        for ci in range(4):
                            nc.tensor.matmul(
                                ps[:], w3[ci][:, ds(128 * co, 128)],
                                h2b[ci][:, b, ds(512 * half, 512)],
                                start=(ci == 0), stop=(ci == 3),
                            )
                        sc = tmp.tile([128, 512], F32, tag="scr")
                        nc.vector.tensor_reduce(sumb[co][:, ds(b * 2 + half, 1)], ps[:], AX, ADD)
                        nc.scalar.square(sc[:], ps[:])
                        nc.vector.tensor_reduce(sqb[co][:, ds(b * 2 + half, 1)], sc[:], AX, ADD)
                        nc.scalar.activation(h3[co][:, b, ds(512 * half, 512)], ps[:], COPY)
            for i in range(4):
                nc.gpsimd.dma_start(out=h3_d[i], in_=h3[i][:].rearrange("p b n -> p (b n)"))
                nc.gpsimd.dma_start(out=st_d[i, :, :, 0], in_=sumb[i][:])
                nc.gpsimd.dma_start(out=st_d[i, :, :, 1], in_=sqb[i][:])
    nc.compile()
    return nc


def _build_l3():
    nc = bacc.Bacc("TRN2", target_bir_lowering=False, debug=False, num_devices=N_CORES)
    h3_d = nc.dram_tensor("h3", [4, 128, NB * HW], BF16, kind="ExternalInput").ap()
    s3_d = nc.dram_tensor("s3", [4, 128, 2], F32, kind="ExternalInput").ap()
    w4_d = nc.dram_tensor("w4", [MID, 9 * C], BF16, kind="ExternalInput").ap()
    bo_d = nc.dram_tensor("bo", [C, 1], F32, kind="ExternalInput").ap()
    cwT_d = nc.dram_tensor("cwT", [8, 128, C], F32, kind="ExternalInput").ap()
    cb_d = nc.dram_tensor("cb", [C, 1], F32, kind="ExternalInput").ap()
    x_d = nc.dram_tensor("x", [NB, C, HW], F32, kind="ExternalInput").ap()
    z_d = nc.dram_tensor("z", [NB, C, HW], F32, kind="ExternalOutput").ap()
    wt_d = nc.dram_tensor("wt", [C, NB * C], F32, kind="ExternalOutput").ap()

    with TileContext(nc) as tc:
        with (
            tc.tile_pool(name="consts", bufs=1) as consts,
            tc.tile_pool(name="act", bufs=1) as act,
            tc.tile_pool(name="ps", bufs=2, space="PSUM") as psum,
        ):
            ident = consts.tile([128, 128], F32)
            make_identity(nc, ident)
            w4 = [consts.tile([128, 9, C], BF16, tag=f"w4_{i}", name=f"w4_{i}") for i in range(4)]
            for i in range(4):
                nc.gpsimd.dma_start(
                    out=w4[i][:],
                    in_=w4_d[ds(128 * i, 128)].rearrange("c (t o) -> c t o", t=9),
                )
            cwT = consts.tile([128, 8, C], F32)
            nc.gpsimd.dma_start(out=cwT[:], in_=cwT_d.rearrange("k p c -> p k c"))
            bo = consts.tile([C, 1], F32)
            nc.gpsimd.dma_start(out=bo[:], in_=bo_d)
            cb = consts.tile([C, 1], F32)
            nc.gpsimd.dma_start(out=cb[:], in_=cb_d)
            s3 = consts.tile([128, 4, 2], F32)
            nc.gpsimd.dma_start(out=s3[:], in_=s3_d.rearrange("k p c -> p k c"))
            x_sb = act.tile([C, NB, HW], F32)
            nc.gpsimd.dma_start(out=x_sb[:], in_=x_d.rearrange("b c n -> c b n"))

            h3p = [act.tile([128, NB, PW, PW], BF16, tag=f"h3p{i}", name=f"h3p{i}") for i in range(4)]
            for i in range(4):
                nc.vector.memset(h3p[i][:], 0.0)
                hin = act.tile([128, NB, HW], BF16, tag=f"hin{i}")
                nc.gpsimd.dma_start(
                    out=hin[:].rearrange("p b n -> p (b n)"), in_=h3_d[i]
                )
                for b in range(NB):
                    nc.scalar.activation(
                        h3p[i][:, b, ds(1, 32), ds(1, 32)],
                        hin[:, b].rearrange("p (h w) -> p h w", h=32),
                        RELU, bias=s3[:, i, ds(1, 1)], scale=s3[:, i, ds(0, 1)],
                    )
            # conv_out + bias -> wmap fp32
            wmap = act.tile([C, NB, HW], F32)

            def wr4(co, b, half, ps):
                nc.scalar.activation(
                    wmap[:, b, ds(512 * half, 512)],
                    ps[:].rearrange("p a b -> p (a b)"), IDENT, bias=bo[:],
                )
            _conv3x3(nc, act, psum, w4, h3p, wr4, 1, 4, ps_bufs=2)

            # transpose wmap -> wmapT, converter matmuls, z matmuls
            wmapT = act.tile([128, NB, 8, C], F32)
            wt_sb = act.tile([C, NB, C], F32)
            z_sb = act.tile([C, NB, HW], F32)
            for b in range(NB):
                for j in range(8):
                    tp = psum.tile([128, C], F32, tag="tp", name="tp", bufs=2)
                    nc.tensor.transpose(tp[:], wmap[:, b, ds(128 * j, 128)], ident[ds(0, 96), ds(0, 96)])
                    nc.vector.tensor_copy(wmapT[:, b, j, :], tp[:])
                wt_ps = psum.tile([C, C], F32, tag="wtps", name="wtps", bufs=2)
                for j in range(8):
                    nc.tensor.matmul(
                        wt_ps[:], cwT[:, j, :], wmapT[:, b, j, :],
                        start=(j == 0), stop=(j == 7),
                    )
                nc.vector.tensor_scalar_add(wt_sb[:, b, :], wt_ps[:], cb[:])
                for half in range(2):
                    zp = psum.tile([C, 512], F32, tag="zps", name="zps", bufs=2)
                    nc.tensor.matmul(
                        zp[:], wt_sb[:, b, :], x_sb[:, b, ds(512 * half, 512)],
                        start=True, stop=True,
                    )
                    nc.vector.tensor_copy(z_sb[:, b, ds(512 * half, 512)], zp[:])
            nc.gpsimd.dma_start(out=z_d.rearrange("b c n -> c b n"), in_=z_sb[:])
            nc.gpsimd.dma_start(out=wt_d, in_=wt_sb[:].rearrange("p b n -> p (b n)"))
    nc.compile()
    return nc


def _get(name, builder):
    if name not in _cache:
        _cache[name] = builder()
    return _cache[name]


def kernel(x, x_cond, sldj, g1, b1, w_in, w_mid1, g2, b2, w_mid2, g3, b3,
           w_out, b_out, conv_w, conv_b):
    x = np.asarray(x, np.float32)
    x_cond = np.asarray(x_cond, np.float32)
    cores = list(range(N_CORES))

    # host: bn1 stats (on raw input) + weight layout prep
    m1 = x_cond.mean(axis=(0, 2, 3))
    v1 = x_cond.var(axis=(0, 2, 3))
    sc1 = np.asarray(g1) / np.sqrt(v1 + EPS)
    sh1 = np.asarray(b1) - m1 * sc1
    s1 = np.stack([sc1, sh1], 1).astype(np.float32)          # [C,2]
    w1 = np.ascontiguousarray(
        np.asarray(w_in).transpose(1, 2, 3, 0).reshape(C, 9 * MID)).astype(BF)
    w2 = np.ascontiguousarray(
        np.asarray(w_mid1).transpose(1, 2, 3, 0).reshape(MID, 9 * MID)).astype(BF)
    w3 = np.ascontiguousarray(np.asarray(w_mid2)[:, :, 0, 0].T).astype(BF)
    w4 = np.ascontiguousarray(
        np.asarray(w_out).transpose(1, 2, 3, 0).reshape(MID, 9 * C)).astype(BF)
    cwT = np.ascontiguousarray(np.asarray(conv_w).T.reshape(8, 128, C)).astype(np.float32)

    xcs = x_cond.reshape(N_CORES, NB, C, HW)
    xs = x.reshape(N_CORES, NB, C, HW)

    nc1 = _get("l1", _build_l1)
    maps = [{"xc": np.ascontiguousarray(xcs[i]), "s1": s1, "w1": w1, "w2": w2}
            for i in range(N_CORES)]
    r1 = run_bass_kernel_spmd(nc1, maps, cores).results

    # merge bn2 stats on host
    st = sum(np.asarray(r["st1"], np.float64).sum(axis=2) for r in r1)  # [4,128,2]
    n = N_CORES * NB * HW
    mean = st[..., 0] / n
    var = st[..., 1] / n - mean ** 2
    sc2 = np.asarray(g2).reshape(4, 128) / np.sqrt(var + EPS)
    sh2 = np.asarray(b2).reshape(4, 128) - mean * sc2
    s2 = np.stack([sc2, sh2], -1).astype(np.float32)  # [4,128,2]

    nc2 = _get("l2", _build_l2)
    maps = [{"h2": r1[i]["h2"], "s2": s2, "w3": w3} for i in range(N_CORES)]
    r2 = run_bass_kernel_spmd(nc2, maps, cores).results

    st = sum(np.asarray(r["st2"], np.float64).sum(axis=2) for r in r2)
    mean = st[..., 0] / n
    var = st[..., 1] / n - mean ** 2
    sc3 = np.asarray(g3).reshape(4, 128) / np.sqrt(var + EPS)
    sh3 = np.asarray(b3).reshape(4, 128) - mean * sc3
    s3 = np.stack([sc3, sh3], -1).astype(np.float32)

    nc3 = _get("l3", _build_l3)
    maps = [{"h3": r2[i]["h3"], "s3": s3, "w4": w4,
             "bo": np.asarray(b_out, np.float32).reshape(C, 1),
             "cwT": cwT,
             "cb": np.asarray(conv_b, np.float32).reshape(C, 1),
             "x": np.ascontiguousarray(xs[i])} for i in range(N_CORES)]
    r3 = run_bass_kernel_spmd(nc3, maps, cores).results

    z = np.concatenate(
        [np.asarray(r["z"], np.float32).reshape(NB, C, H, W) for r in r3], 0)
    ldj = []
    for r in r3:
        wt = np.asarray(r["wt"], np.float32).reshape(C, NB, C)
        for b in range(NB):
            Wb = wt[:, b, :].T  # [c, d]
            ldj.append(np.linalg.slogdet(Wb)[1] * HW)
    sldj_out = (np.asarray(sldj, np.float32) + np.asarray(ldj, np.float32))
    return z, sldj_out


# revision 9
# speedup vs baseline: 1.3862x; 1.3862x over previous
import sys
sys.path.insert(0, "/opt/trn_rl_repo")
import numpy as np
import ml_dtypes
import concourse.mybir as mybir
import concourse.bacc as bacc
from concourse.bass import ds
from concourse.tile import TileContext
from concourse.bass_utils import run_bass_kernel_spmd
from concourse.masks import make_identity

F32 = mybir.dt.float32
BF16 = mybir.dt.bfloat16
BF = ml_dtypes.bfloat16
N_CORES = 8
NB = 4          # samples per core
C = 96
MID = 512
H = W = 32
HW = 1024
PW = 34
PHW = PW * PW   # 1156
EPS = 1e-5
AX = mybir.AxisListType.X
ADD = mybir.AluOpType.add
RELU = mybir.ActivationFunctionType.Relu
COPY = mybir.ActivationFunctionType.Copy
IDENT = mybir.ActivationFunctionType.Identity

_cache = {}
last_exec_times = []


def _conv3x3(nc, pool, psum, lhsT_tiles, src_pad, dst_write, cout_blocks, n_cin, ps_bufs=6):
    """3x3 conv: src_pad tiles [P,NB,34,34]; lhsT_tiles[ci][:, t, co_sl] bf16.
    dst_write(co_blk, b, half, ps) consumes PSUM [P_out, 16, 32]."""
    for co in range(cout_blocks):
        for b in range(NB):
            for half in range(2):
                pout = 96 if cout_blocks == 1 else 128
                ps = psum.tile([pout, 16, 32], F32, tag="convps", name="convps", bufs=ps_bufs)
                first = True
                for ci in range(n_cin):
                    for t in range(9):
                        ky, kx = t // 3, t % 3
                        rhs = src_pad[ci][:, b, ds(16 * half + ky, 16), ds(kx, 32)]
                        co_sl = ds(co * 128, 128) if cout_blocks > 1 else ds(0, 96)
                        nc.tensor.matmul(
                            ps[:], lhsT_tiles[ci][:, t, co_sl], rhs,
                            start=first, stop=(ci == n_cin - 1 and t == 8),
                        )
                        first = False
                dst_write(co, b, half, ps)


def _stats(nc, pool, ps, sumb, sqb, idx, scratch):
    psf = ps[:].rearrange("p a b -> p (a b)")
    scf = scratch[:].rearrange("p a b -> p (a b)")
    nc.vector.tensor_reduce(sumb[:, ds(idx, 1)], psf, AX, ADD)
    nc.scalar.square(scf, psf)
    nc.vector.tensor_reduce(sqb[:, ds(idx, 1)], scf, AX, ADD)


def _build_l1():
    nc = bacc.Bacc("TRN2", target_bir_lowering=False, debug=False, num_devices=N_CORES)
    xc_d = nc.dram_tensor("xc", [NB, C, HW], F32, kind="ExternalInput").ap()
    s1_d = nc.dram_tensor("s1", [C, 2], F32, kind="ExternalInput").ap()
    w1_d = nc.dram_tensor("w1", [C, 9 * MID], BF16, kind="ExternalInput").ap()
    w2_d = nc.dram_tensor("w2", [MID, 9 * MID], BF16, kind="ExternalInput").ap()
    h2_d = nc.dram_tensor("h2", [4, 128, NB * HW], BF16, kind="ExternalOutput").ap()
    st_d = nc.dram_tensor("st1", [4, 128, 8, 2], F32, kind="ExternalOutput").ap()

    with TileContext(nc) as tc:
        with (
            tc.tile_pool(name="consts", bufs=1) as consts,
            tc.tile_pool(name="act", bufs=1) as act,
            tc.tile_pool(name="ps", bufs=2, space="PSUM") as psum,
            tc.tile_pool(name="tmp", bufs=4) as tmp,
        ):
            # weights
            w1 = consts.tile([C, 9, MID], BF16)
            nc.gpsimd.dma_start(out=w1[:], in_=w1_d.rearrange("c (t o) -> c t o", t=9))
            w2 = [consts.tile([128, 9, MID], BF16, tag=f"w2_{i}", name=f"w2_{i}") for i in range(4)]
            for i in range(4):
                nc.gpsimd.dma_start(
                    out=w2[i][:],
                    in_=w2_d[ds(128 * i, 128)].rearrange("c (t o) -> c t o", t=9),
                )
            s1 = consts.tile([C, 2], F32)
            nc.gpsimd.dma_start(out=s1[:], in_=s1_d)
            # x_cond -> bn1 -> padded bf16
            xc = act.tile([C, NB, HW], F32)
            nc.gpsimd.dma_start(out=xc[:], in_=xc_d.rearrange("b c n -> c b n"))
            xcp = act.tile([C, NB, PW, PW], BF16)
            nc.vector.memset(xcp[:], 0.0)
            for b in range(NB):
                nc.scalar.activation(
                    xcp[:, b, ds(1, 32), ds(1, 32)],
                    xc[:, b].rearrange("p (h w) -> p h w", h=32),
                    IDENT, bias=s1[:, ds(1, 1)], scale=s1[:, ds(0, 1)],
                )
            # conv_in + relu -> h1p
            h1p = [act.tile([128, NB, PW, PW], BF16, tag=f"h1_{i}", name=f"h1_{i}") for i in range(4)]
            for i in range(4):
                nc.vector.memset(h1p[i][:], 0.0)

            def wr1(co, b, half, ps):
                nc.scalar.activation(
                    h1p[co][:, b, ds(1 + 16 * half, 16), ds(1, 32)], ps[:], RELU
                )
            _conv3x3(nc, act, psum, [w1], [xcp], wr1, 4, 1)

            # conv_mid1 -> h2 + stats
            sumb = [act.tile([128, 8], F32, tag=f"su{i}", name=f"su{i}") for i in range(4)]
            sqb = [act.tile([128, 8], F32, tag=f"sq{i}", name=f"sq{i}") for i in range(4)]
            h2 = [act.tile([128, NB, HW], BF16, tag=f"h2_{i}", name=f"h2_{i}") for i in range(4)]
            scratch = tmp.tile([128, 16, 32], F32, tag="scr")

            def wr2(co, b, half, ps):
                sc = tmp.tile([128, 16, 32], F32, tag="scr")
                _stats(nc, act, ps, sumb[co], sqb[co], b * 2 + half, sc)
                nc.scalar.activation(
                    h2[co][:, b, ds(512 * half, 512)],
                    ps[:].rearrange("p a b -> p (a b)"), COPY,
                )
            _conv3x3(nc, act, psum, w2, h1p, wr2, 4, 4)

            for i in range(4):
                nc.gpsimd.dma_start(out=h2_d[i], in_=h2[i][:].rearrange("p b n -> p (b n)"))
                nc.gpsimd.dma_start(out=st_d[i, :, :, 0], in_=sumb[i][:])
                nc.gpsimd.dma_start(out=st_d[i, :, :, 1], in_=sqb[i][:])
    nc.compile()
    return nc


def _build_l2():
    nc = bacc.Bacc("TRN2", target_bir_lowering=False, debug=False, num_devices=N_CORES)
    h2_d = nc.dram_tensor("h2", [4, 128, NB * HW], BF16, kind="ExternalInput").ap()
    s2_d = nc.dram_tensor("s2", [4, 128, 2], F32, kind="ExternalInput").ap()
    w3_d = nc.dram_tensor("w3", [MID, MID], BF16, kind="ExternalInput").ap()
    h3_d = nc.dram_tensor("h3", [4, 128, NB * HW], BF16, kind="ExternalOutput").ap()
    st_d = nc.dram_tensor("st2", [4, 128, 8, 2], F32, kind="ExternalOutput").ap()

    with TileContext(nc) as tc:
        with (
            tc.tile_pool(name="consts", bufs=1) as consts,
            tc.tile_pool(name="act", bufs=1) as act,
            tc.tile_pool(name="ps", bufs=2, space="PSUM") as psum,
            tc.tile_pool(name="tmp", bufs=4) as tmp,
        ):
            w3 = [consts.tile([128, MID], BF16, tag=f"w3_{i}", name=f"w3_{i}") for i in range(4)]
            for i in range(4):
                nc.gpsimd.dma_start(out=w3[i][:], in_=w3_d[ds(128 * i, 128)])
            s2 = consts.tile([128, 4, 2], F32)
            nc.gpsimd.dma_start(out=s2[:], in_=s2_d.rearrange("k p c -> p k c"))
            h2b = [act.tile([128, NB, HW], BF16, tag=f"h2b{i}", name=f"h2b{i}") for i in range(4)]
            for i in range(4):
                hin = act.tile([128, NB * HW], BF16, tag=f"hin{i}")
                nc.gpsimd.dma_start(out=hin[:], in_=h2_d[i])
                nc.scalar.activation(
                    h2b[i][:].rearrange("p b n -> p (b n)"), hin[:],
                    RELU, bias=s2[:, i, ds(1, 1)], scale=s2[:, i, ds(0, 1)],
                )
            sumb = [act.tile([128, 8], F32, tag=f"su{i}", name=f"su{i}") for i in range(4)]
            sqb = [act.tile([128, 8], F32, tag=f"sq{i}", name=f"sq{i}") for i in range(4)]
            h3 = [act.tile([128, NB, HW], BF16, tag=f"h3_{i}", name=f"h3_{i}") for i in range(4)]
            for co in range(4):
                for b in range(NB):
                    for half in range(2):
                        ps = psum.tile([128, 512], F32, tag="ps", name="ps", bufs=6)
                        for ci in range(4):
                            nc.tensor.matmul(
                                ps[:], w3[ci][:, ds(128 * co, 128)],
                                h2b[ci][:, b, ds(512 * half, 512)],
                                start=(ci == 0), stop=(ci == 3),
                            )
                        sc = tmp.tile([128, 512], F32, tag="scr")
                        nc.vector.tensor_reduce(sumb[co][:, ds(b * 2 + half, 1)], ps[:], AX, ADD)
                        nc.scalar.square(sc[:], ps[:])
                        nc.vector.tensor_reduce(sqb[co][:, ds(b * 2 + half, 1)], sc[:], AX, ADD)
                        nc.scalar.activation(h3[co][:, b, ds(512 * half, 512)], ps[:], COPY)
            for i in range(4):
                nc.gpsimd.dma_start(out=h3_d[i], in_=h3[i][:].rearrange("p b n -> p (b n)"))
                nc.gpsimd.dma_start(out=st_d[i, :, :, 0], in_=sumb[i][:])
                nc.gpsimd.dma_start(out=st_d[i, :, :, 1], in_=sqb[i][:])
    nc.compile()
    return nc


def _build_l3():
    nc = bacc.Bacc("TRN2", target_bir_lowering=False, debug=False, num_devices=N_CORES)
    h3_d = nc.dram_tensor("h3", [4, 128, NB * HW], BF16, kind="ExternalInput").ap()
    s3_d = nc.dram_tensor("s3", [4, 128, 2], F32, kind="ExternalInput").ap()
    w4_d = nc.dram_tensor("w4", [MID, 9 * C], BF16, kind="ExternalInput").ap()
    bo_d = nc.dram_tensor("bo", [C, 1], F32, kind="ExternalInput").ap()
    cwT_d = nc.dram_tensor("cwT", [8, 128, C], F32, kind="ExternalInput").ap()
    cb_d = nc.dram_tensor("cb", [C, 1], F32, kind="ExternalInput").ap()
    x_d = nc.dram_tensor("x", [NB, C, HW], F32, kind="ExternalInput").ap()
    z_d = nc.dram_tensor("z", [NB, C, HW], F32, kind="ExternalOutput").ap()
    wt_d = nc.dram_tensor("wt", [C, NB * C], F32, kind="ExternalOutput").ap()

    with TileContext(nc) as tc:
        with (
            tc.tile_pool(name="consts", bufs=1) as consts,
            tc.tile_pool(name="act", bufs=1) as act,
            tc.tile_pool(name="ps", bufs=2, space="PSUM") as psum,
        ):
            ident = consts.tile([128, 128], F32)
            make_identity(nc, ident)
            w4 = [consts.tile([128, 9, C], BF16, tag=f"w4_{i}", name=f"w4_{i}") for i in range(4)]
            for i in range(4):
                nc.gpsimd.dma_start(
                    out=w4[i][:],
                    in_=w4_d[ds(128 * i, 128)].rearrange("c (t o) -> c t o", t=9),
                )
            cwT = consts.tile([128, 8, C], F32)
            nc.gpsimd.dma_start(out=cwT[:], in_=cwT_d.rearrange("k p c -> p k c"))
            bo = consts.tile([C, 1], F32)
            nc.gpsimd.dma_start(out=bo[:], in_=bo_d)
            cb = consts.tile([C, 1], F32)
            nc.gpsimd.dma_start(out=cb[:], in_=cb_d)
            s3 = consts.tile([128, 4, 2], F32)
            nc.gpsimd.dma_start(out=s3[:], in_=s3_d.rearrange("k p c -> p k c"))
            x_sb = act.tile([C, NB, HW], F32)
            nc.gpsimd.dma_start(out=x_sb[:], in_=x_d.rearrange("b c n -> c b n"))

            h3p = [act.tile([128, NB, PW, PW], BF16, tag=f"h3p{i}", name=f"h3p{i}") for i in range(4)]
            for i in range(4):
                nc.vector.memset(h3p[i][:], 0.0)
                hin = act.tile([128, NB, HW], BF16, tag=f"hin{i}")
                nc.gpsimd.dma_start(
                    out=hin[:].rearrange("p b n -> p (b n)"), in_=h3_d[i]
                )
                for b in range(NB):
                    nc.scalar.activation(
                        h3p[i][:, b, ds(1, 32), ds(1, 32)],
                        hin[:, b].rearrange("p (h w) -> p h w", h=32),
                        RELU, bias=s3[:, i, ds(1, 1)], scale=s3[:, i, ds(0, 1)],
                    )
            # conv_out + bias -> wmap fp32
            wmap = act.tile([C, NB, HW], F32)

            def wr4(co, b, half, ps):
                nc.scalar.activation(
                    wmap[:, b, ds(512 * half, 512)],
                    ps[:].rearrange("p a b -> p (a b)"), IDENT, bias=bo[:],
                )
            _conv3x3(nc, act, psum, w4, h3p, wr4, 1, 4, ps_bufs=2)

            # transpose wmap -> wmapT, converter matmuls, z matmuls
            wmapT = act.tile([128, NB, 8, C], F32)
            wt_sb = act.tile([C, NB, C], F32)
            z_sb = act.tile([C, NB, HW], F32)
            for b in range(NB):
                for j in range(8):
                    tp = psum.tile([128, C], F32, tag="tp", name="tp", bufs=2)
                    nc.tensor.transpose(tp[:], wmap[:, b, ds(128 * j, 128)], ident[ds(0, 96), ds(0, 96)])
                    nc.vector.tensor_copy(wmapT[:, b, j, :], tp[:])
                wt_ps = psum.tile([C, C], F32, tag="wtps", name="wtps", bufs=2)
                for j in range(8):
                    nc.tensor.matmul(
                        wt_ps[:], cwT[:, j, :], wmapT[:, b, j, :],
                        start=(j == 0), stop=(j == 7),
                    )
                nc.vector.tensor_scalar_add(wt_sb[:, b, :], wt_ps[:], cb[:])
                for half in range(2):
                    zp = psum.tile([C, 512], F32, tag="zps", name="zps", bufs=2)
                    nc.tensor.matmul(
                        zp[:], wt_sb[:, b, :], x_sb[:, b, ds(512 * half, 512)],
                        start=True, stop=True,
                    )
                    nc.vector.tensor_copy(z_sb[:, b, ds(512 * half, 512)], zp[:])
            nc.gpsimd.dma_start(out=z_d.rearrange("b c n -> c b n"), in_=z_sb[:])
            nc.gpsimd.dma_start(out=wt_d, in_=wt_sb[:].rearrange("p b n -> p (b n)"))
    nc.compile()
    return nc


def _get(name, builder):
    if name not in _cache:
        _cache[name] = builder()
    return _cache[name]


def kernel(x, x_cond, sldj, g1, b1, w_in, w_mid1, g2, b2, w_mid2, g3, b3,
           w_out, b_out, conv_w, conv_b):
    x = np.asarray(x, np.float32)
    x_cond = np.asarray(x_cond, np.float32)
    cores = list(range(N_CORES))
    last_exec_times.clear()

    # host: bn1 stats (on raw input) + weight layout prep
    m1 = x_cond.mean(axis=(0, 2, 3))
    v1 = x_cond.var(axis=(0, 2, 3))
    sc1 = np.asarray(g1) / np.sqrt(v1 + EPS)
    sh1 = np.asarray(b1) - m1 * sc1
    s1 = np.stack([sc1, sh1], 1).astype(np.float32)          # [C,2]
    w1 = np.ascontiguousarray(
        np.asarray(w_in).transpose(1, 2, 3, 0).reshape(C, 9 * MID)).astype(BF)
    w2 = np.ascontiguousarray(
        np.asarray(w_mid1).transpose(1, 2, 3, 0).reshape(MID, 9 * MID)).astype(BF)
    w3 = np.ascontiguousarray(np.asarray(w_mid2)[:, :, 0, 0].T).astype(BF)
    w4 = np.ascontiguousarray(
        np.asarray(w_out).transpose(1, 2, 3, 0).reshape(MID, 9 * C)).astype(BF)
    cwT = np.ascontiguousarray(np.asarray(conv_w).T.reshape(8, 128, C)).astype(np.float32)

    xcs = x_cond.reshape(N_CORES, NB, C, HW)
    xs = x.reshape(N_CORES, NB, C, HW)

    nc1 = _get("l1", _build_l1)
    maps = [{"xc": np.ascontiguousarray(xcs[i]), "s1": s1, "w1": w1, "w2": w2}
            for i in range(N_CORES)]
    _r = run_bass_kernel_spmd(nc1, maps, cores)
    last_exec_times.append(_r.exec_time_ns)
    r1 = _r.results

    # merge bn2 stats on host
    st = sum(np.asarray(r["st1"], np.float64).sum(axis=2) for r in r1)  # [4,128,2]
    n = N_CORES * NB * HW
    mean = st[..., 0] / n
    var = st[..., 1] / n - mean ** 2
    sc2 = np.asarray(g2).reshape(4, 128) / np.sqrt(var + EPS)
    sh2 = np.asarray(b2).reshape(4, 128) - mean * sc2
    s2 = np.stack([sc2, sh2], -1).astype(np.float32)  # [4,128,2]

    nc2 = _get("l2", _build_l2)
    maps = [{"h2": r1[i]["h2"], "s2": s2, "w3": w3} for i in range(N_CORES)]
    _r = run_bass_kernel_spmd(nc2, maps, cores)
    last_exec_times.append(_r.exec_time_ns)
    r2 = _r.results

    st = sum(np.asarray(r["st2"], np.float64).sum(axis=2) for r in r2)
    mean = st[..., 0] / n
    var = st[..., 1] / n - mean ** 2
    sc3 = np.asarray(g3).reshape(4, 128) / np.sqrt(var + EPS)
    sh3 = np.asarray(b3).reshape(4, 128) - mean * sc3
    s3 = np.stack([sc3, sh3], -1).astype(np.float32)

    nc3 = _get("l3", _build_l3)
    maps = [{"h3": r2[i]["h3"], "s3": s3, "w4": w4,
             "bo": np.asarray(b_out, np.float32).reshape(C, 1),
             "cwT": cwT,
             "cb": np.asarray(conv_b, np.float32).reshape(C, 1),
             "x": np.ascontiguousarray(xs[i])} for i in range(N_CORES)]
    _r = run_bass_kernel_spmd(nc3, maps, cores)
    last_exec_times.append(_r.exec_time_ns)
    r3 = _r.results

    z = np.concatenate(
        [np.asarray(r["z"], np.float32).reshape(NB, C, H, W) for r in r3], 0)
    ldj = []
    for r in r3:
        wt = np.asarray(r["wt"], np.float32).reshape(C, NB, C)
        for b in range(NB):
            Wb = wt[:, b, :].T  # [c, d]
            ldj.append(np.linalg.slogdet(Wb)[1] * HW)
    sldj_out = (np.asarray(sldj, np.float32) + np.asarray(ldj, np.float32))
    return z, sldj_out


# revision 10
# speedup vs baseline: 1.4208x; 1.0250x over previous
import sys
sys.path.insert(0, "/opt/trn_rl_repo")
import numpy as np
import ml_dtypes
import concourse.mybir as mybir
import concourse.bacc as bacc
from concourse.bass import ds
from concourse.tile import TileContext
from concourse.bass_utils import run_bass_kernel_spmd
from concourse.masks import make_identity

F32 = mybir.dt.float32
BF16 = mybir.dt.bfloat16
BF = ml_dtypes.bfloat16
N_CORES = 8
NB = 4          # samples per core
C = 96
MID = 512
H = W = 32
HW = 1024
PW = 34
PHW = PW * PW   # 1156
EPS = 1e-5
AX = mybir.AxisListType.X
ADD = mybir.AluOpType.add
RELU = mybir.ActivationFunctionType.Relu
COPY = mybir.ActivationFunctionType.Copy
IDENT = mybir.ActivationFunctionType.Identity

_cache = {}
last_exec_times = []


def _conv3x3(nc, pool, psum, lhsT_tiles, src_pad, dst_write, cout_blocks, n_cin, ps_bufs=6):
    """3x3 conv: src_pad tiles [P,NB,34,34]; lhsT_tiles[ci][:, t, co_sl] bf16.
    dst_write(co_blk, b, half, ps) consumes PSUM [P_out, 16, 32]."""
    for co in range(cout_blocks):
        for b in range(NB):
            for half in range(2):
                pout = 96 if cout_blocks == 1 else 128
                ps = psum.tile([pout, 16, 32], F32, tag="convps", name="convps", bufs=ps_bufs)
                first = True
                for ci in range(n_cin):
                    for t in range(9):
                        ky, kx = t // 3, t % 3
                        rhs = src_pad[ci][:, b, ds(16 * half + ky, 16), ds(kx, 32)]
                        co_sl = ds(co * 128, 128) if cout_blocks > 1 else ds(0, 96)
                        nc.tensor.matmul(
                            ps[:], lhsT_tiles[ci][:, t, co_sl], rhs,
                            start=first, stop=(ci == n_cin - 1 and t == 8),
                        )
                        first = False
                dst_write(co, b, half, ps)


def _stats(nc, pool, ps, sumb, sqb, idx, scratch):
    psf = ps[:].rearrange("p a b -> p (a b)")
    scf = scratch[:].rearrange("p a b -> p (a b)")
    nc.vector.tensor_reduce(sumb[:, ds(idx, 1)], psf, AX, ADD)
    nc.scalar.square(scf, psf)
    nc.vector.tensor_reduce(sqb[:, ds(idx, 1)], scf, AX, ADD)


def _build_l1():
    nc = bacc.Bacc("TRN2", target_bir_lowering=False, debug=False, num_devices=N_CORES)
    xc_d = nc.dram_tensor("xc", [NB, C, HW], F32, kind="ExternalInput").ap()
    s1_d = nc.dram_tensor("s1", [C, 2], F32, kind="ExternalInput").ap()
    w1_d = nc.dram_tensor("w1", [C, 9 * MID], BF16, kind="ExternalInput").ap()
    w2_d = nc.dram_tensor("w2", [MID, 9 * MID], BF16, kind="ExternalInput").ap()
    h2_d = nc.dram_tensor("h2", [4, 128, NB * HW], BF16, kind="ExternalOutput").ap()
    st_d = nc.dram_tensor("st1", [4, 128, 8, 2], F32, kind="ExternalOutput").ap()

    with TileContext(nc) as tc:
        with (
            tc.tile_pool(name="consts", bufs=1) as consts,
            tc.tile_pool(name="act", bufs=1) as act,
            tc.tile_pool(name="ps", bufs=2, space="PSUM") as psum,
            tc.tile_pool(name="tmp", bufs=4) as tmp,
        ):
            # weights
            w1 = consts.tile([C, 9, MID], BF16)
            nc.gpsimd.dma_start(out=w1[:], in_=w1_d.rearrange("c (t o) -> c t o", t=9))
            w2 = [consts.tile([128, 9, MID], BF16, tag=f"w2_{i}", name=f"w2_{i}") for i in range(4)]
            for i in range(4):
                nc.gpsimd.dma_start(
                    out=w2[i][:],
                    in_=w2_d[ds(128 * i, 128)].rearrange("c (t o) -> c t o", t=9),
                )
            s1 = consts.tile([C, 2], F32)
            nc.gpsimd.dma_start(out=s1[:], in_=s1_d)
            # x_cond -> bn1 -> padded bf16
            xc = act.tile([C, NB, HW], F32)
            nc.gpsimd.dma_start(out=xc[:], in_=xc_d.rearrange("b c n -> c b n"))
            xcp = act.tile([C, NB, PW, PW], BF16)
            nc.vector.memset(xcp[:], 0.0)
            for b in range(NB):
                nc.scalar.activation(
                    xcp[:, b, ds(1, 32), ds(1, 32)],
                    xc[:, b].rearrange("p (h w) -> p h w", h=32),
                    IDENT, bias=s1[:, ds(1, 1)], scale=s1[:, ds(0, 1)],
                )
            # conv_in + relu -> h1p
            h1p = [act.tile([128, NB, PW, PW], BF16, tag=f"h1_{i}", name=f"h1_{i}") for i in range(4)]
            for i in range(4):
                nc.vector.memset(h1p[i][:], 0.0)

            def wr1(co, b, half, ps):
                nc.scalar.activation(
                    h1p[co][:, b, ds(1 + 16 * half, 16), ds(1, 32)], ps[:], RELU
                )
            _conv3x3(nc, act, psum, [w1], [xcp], wr1, 4, 1)

            # conv_mid1 -> h2 + stats
            sumb = [act.tile([128, 8], F32, tag=f"su{i}", name=f"su{i}") for i in range(4)]
            sqb = [act.tile([128, 8], F32, tag=f"sq{i}", name=f"sq{i}") for i in range(4)]
            h2 = [act.tile([128, NB, HW], BF16, tag=f"h2_{i}", name=f"h2_{i}") for i in range(4)]
            scratch = tmp.tile([128, 16, 32], F32, tag="scr")

            def wr2(co, b, half, ps):
                sc = tmp.tile([128, 16, 32], F32, tag="scr")
                _stats(nc, act, ps, sumb[co], sqb[co], b * 2 + half, sc)
                nc.scalar.activation(
                    h2[co][:, b, ds(512 * half, 512)],
                    ps[:].rearrange("p a b -> p (a b)"), COPY,
                )
            _conv3x3(nc, act, psum, w2, h1p, wr2, 4, 4)

            for i in range(4):
                nc.gpsimd.dma_start(out=h2_d[i], in_=h2[i][:].rearrange("p b n -> p (b n)"))
                nc.gpsimd.dma_start(out=st_d[i, :, :, 0], in_=sumb[i][:])
                nc.gpsimd.dma_start(out=st_d[i, :, :, 1], in_=sqb[i][:])
    nc.compile()
    return nc


def _build_l2():
    nc = bacc.Bacc("TRN2", target_bir_lowering=False, debug=False, num_devices=N_CORES)
    h2_d = nc.dram_tensor("h2", [4, 128, NB * HW], BF16, kind="ExternalInput").ap()
    s2_d = nc.dram_tensor("s2", [4, 128, 2], F32, kind="ExternalInput").ap()
    w3_d = nc.dram_tensor("w3", [MID, MID], BF16, kind="ExternalInput").ap()
    h3_d = nc.dram_tensor("h3", [4, 128, NB * HW], BF16, kind="ExternalOutput").ap()
    st_d = nc.dram_tensor("st2", [4, 128, 8, 2], F32, kind="ExternalOutput").ap()

    with TileContext(nc) as tc:
        with (
            tc.tile_pool(name="consts", bufs=1) as consts,
            tc.tile_pool(name="act", bufs=1) as act,
            tc.tile_pool(name="ps", bufs=2, space="PSUM") as psum,
            tc.tile_pool(name="tmp", bufs=4) as tmp,
        ):
            w3 = [consts.tile([128, MID], BF16, tag=f"w3_{i}", name=f"w3_{i}") for i in range(4)]
            for i in range(4):
                nc.gpsimd.dma_start(out=w3[i][:], in_=w3_d[ds(128 * i, 128)])
            s2 = consts.tile([128, 4, 2], F32)
            nc.gpsimd.dma_start(out=s2[:], in_=s2_d.rearrange("k p c -> p k c"))
            h2b = [act.tile([128, NB, HW], BF16, tag=f"h2b{i}", name=f"h2b{i}") for i in range(4)]
            for i in range(4):
                hin = act.tile([128, NB * HW], BF16, tag=f"hin{i}")
                nc.gpsimd.dma_start(out=hin[:], in_=h2_d[i])
                nc.scalar.activation(
                    h2b[i][:].rearrange("p b n -> p (b n)"), hin[:],
                    RELU, bias=s2[:, i, ds(1, 1)], scale=s2[:, i, ds(0, 1)],
                )
            sumb = [act.tile([128, 8], F32, tag=f"su{i}", name=f"su{i}") for i in range(4)]
            sqb = [act.tile([128, 8], F32, tag=f"sq{i}", name=f"sq{i}") for i in range(4)]
            h3 = [act.tile([128, NB, HW], BF16, tag=f"h3_{i}", name=f"h3_{i}") for i in range(4)]
            for co in range(4):
                for b in range(NB):
                    for half in range(2):
                        ps = psum.tile([128, 512], F32, tag="ps", name="ps", bufs=6)
                        for ci in range(4):
                            nc.tensor.matmul(
                                ps[:], w3[ci][:, ds(128 * co, 128)],
                                h2b[ci][:, b, ds(512 * half, 512)],
                                start=(ci == 0), stop=(ci == 3),
                            )
                        sc = tmp.tile([128, 512], F32, tag="scr")
                        nc.vector.tensor_reduce(sumb[co][:, ds(b * 2 + half, 1)], ps[:], AX, ADD)
                        nc.scalar.square(sc[:], ps[:])
                        nc.vector.tensor_reduce(sqb[co][:, ds(b * 2 + half, 1)], sc[:], AX, ADD)
                        nc.scalar.activation(h3[co][:, b, ds(512 * half, 512)], ps[:], COPY)
            for i in range(4):
                nc.gpsimd.dma_start(out=h3_d[i], in_=h3[i][:].rearrange("p b n -> p (b n)"))
                nc.gpsimd.dma_start(out=st_d[i, :, :, 0], in_=sumb[i][:])
                nc.gpsimd.dma_start(out=st_d[i, :, :, 1], in_=sqb[i][:])
    nc.compile()
    return nc


def _build_l3():
    nc = bacc.Bacc("TRN2", target_bir_lowering=False, debug=False, num_devices=N_CORES)
    h3_d = nc.dram_tensor("h3", [4, 128, NB * HW], BF16, kind="ExternalInput").ap()
    s3_d = nc.dram_tensor("s3", [4, 128, 2], F32, kind="ExternalInput").ap()
    w4_d = nc.dram_tensor("w4", [MID, 9 * C], BF16, kind="ExternalInput").ap()
    bo_d = nc.dram_tensor("bo", [C, 1], F32, kind="ExternalInput").ap()
    cwT_d = nc.dram_tensor("cwT", [8, 128, C], F32, kind="ExternalInput").ap()
    cb_d = nc.dram_tensor("cb", [C, 1], F32, kind="ExternalInput").ap()
    x_d = nc.dram_tensor("x", [NB, C, HW], F32, kind="ExternalInput").ap()
    z_d = nc.dram_tensor("z", [NB, C, HW], F32, kind="ExternalOutput").ap()
    wt_d = nc.dram_tensor("wt", [C, NB * C], F32, kind="ExternalOutput").ap()

    with TileContext(nc) as tc:
        with (
            tc.tile_pool(name="consts", bufs=1) as consts,
            tc.tile_pool(name="act", bufs=1) as act,
            tc.tile_pool(name="ps", bufs=2, space="PSUM") as psum,
        ):
            ident = consts.tile([128, 128], F32)
            make_identity(nc, ident)
            w4 = [consts.tile([128, 9, C], BF16, tag=f"w4_{i}", name=f"w4_{i}") for i in range(4)]
            for i in range(4):
                nc.gpsimd.dma_start(
                    out=w4[i][:],
                    in_=w4_d[ds(128 * i, 128)].rearrange("c (t o) -> c t o", t=9),
                )
            cwT = consts.tile([128, 8, C], F32)
            nc.gpsimd.dma_start(out=cwT[:], in_=cwT_d.rearrange("k p c -> p k c"))
            bo = consts.tile([C, 1], F32)
            nc.gpsimd.dma_start(out=bo[:], in_=bo_d)
            cb = consts.tile([C, 1], F32)
            nc.gpsimd.dma_start(out=cb[:], in_=cb_d)
            s3 = consts.tile([128, 4, 2], F32)
            nc.gpsimd.dma_start(out=s3[:], in_=s3_d.rearrange("k p c -> p k c"))
            x_sb = act.tile([C, NB, HW], F32)
            nc.gpsimd.dma_start(out=x_sb[:], in_=x_d.rearrange("b c n -> c b n"))

            h3p = [act.tile([128, NB, PW, PW], BF16, tag=f"h3p{i}", name=f"h3p{i}") for i in range(4)]
            for i in range(4):
                nc.vector.memset(h3p[i][:], 0.0)
                hin = act.tile([128, NB, HW], BF16, tag=f"hin{i}")
                nc.gpsimd.dma_start(
                    out=hin[:].rearrange("p b n -> p (b n)"), in_=h3_d[i]
                )
                for b in range(NB):
                    nc.scalar.activation(
                        h3p[i][:, b, ds(1, 32), ds(1, 32)],
                        hin[:, b].rearrange("p (h w) -> p h w", h=32),
                        RELU, bias=s3[:, i, ds(1, 1)], scale=s3[:, i, ds(0, 1)],
                    )
            # conv_out + bias -> wmap fp32
            wmap = act.tile([C, NB, HW], F32)

            def wr4(co, b, half, ps):
                nc.scalar.activation(
                    wmap[:, b, ds(512 * half, 512)],
                    ps[:].rearrange("p a b -> p (a b)"), IDENT, bias=bo[:],
                )
            _conv3x3(nc, act, psum, w4, h3p, wr4, 1, 4, ps_bufs=2)

            # transpose wmap -> wmapT, converter matmuls, z matmuls
            wmapT = act.tile([128, NB, 8, C], F32)
            wt_sb = act.tile([C, NB, C], F32)
            z_sb = act.tile([C, NB, HW], F32)
            for b in range(NB):
                for j in range(8):
                    tp = psum.tile([128, C], F32, tag="tp", name="tp", bufs=2)
                    nc.tensor.transpose(tp[:], wmap[:, b, ds(128 * j, 128)], ident[ds(0, 96), ds(0, 96)])
                    nc.vector.tensor_copy(wmapT[:, b, j, :], tp[:])
                wt_ps = psum.tile([C, C], F32, tag="wtps", name="wtps", bufs=2)
                for j in range(8):
                    nc.tensor.matmul(
                        wt_ps[:], cwT[:, j, :], wmapT[:, b, j, :],
                        start=(j == 0), stop=(j == 7),
                    )
                nc.vector.tensor_scalar_add(wt_sb[:, b, :], wt_ps[:], cb[:])
                for half in range(2):
                    zp = psum.tile([C, 512], F32, tag="zps", name="zps", bufs=2)
                    nc.tensor.matmul(
                        zp[:], wt_sb[:, b, :], x_sb[:, b, ds(512 * half, 512)],
                        start=True, stop=True,
                    )
                    nc.vector.tensor_copy(z_sb[:, b, ds(512 * half, 512)], zp[:])
            nc.gpsimd.dma_start(out=z_d.rearrange("b c n -> c b n"), in_=z_sb[:])
            nc.gpsimd.dma_start(out=wt_d, in_=wt_sb[:].rearrange("p b n -> p (b n)"))
    nc.compile()
    return nc


def _get(name, builder):
    if name not in _cache:
        _cache[name] = builder()
    return _cache[name]


def kernel(x, x_cond, sldj, g1, b1, w_in, w_mid1, g2, b2, w_mid2, g3, b3,
           w_out, b_out, conv_w, conv_b):
    x = np.asarray(x, np.float32)
    x_cond = np.asarray(x_cond, np.float32)
    cores = list(range(N_CORES))
    last_exec_times.clear()

    # host: bn1 stats (on raw input) + weight layout prep
    m1 = x_cond.mean(axis=(0, 2, 3))
    v1 = x_cond.var(axis=(0, 2, 3))
    sc1 = np.asarray(g1) / np.sqrt(v1 + EPS)
    sh1 = np.asarray(b1) - m1 * sc1
    s1 = np.stack([sc1, sh1], 1).astype(np.float32)          # [C,2]
    w1 = np.ascontiguousarray(
        np.asarray(w_in).transpose(1, 2, 3, 0).reshape(C, 9 * MID)).astype(BF)
    w2 = np.ascontiguousarray(
        np.asarray(w_mid1).transpose(1, 2, 3, 0).reshape(MID, 9 * MID)).astype(BF)
    w3 = np.ascontiguousarray(np.asarray(w_mid2)[:, :, 0, 0].T).astype(BF)
    w4 = np.ascontiguousarray(
        np.asarray(w_out).transpose(1, 2, 3, 0).reshape(MID, 9 * C)).astype(BF)
    cwT = np.ascontiguousarray(np.asarray(conv_w).T.reshape(8, 128, C)).astype(np.float32)

    xcs = x_cond.reshape(N_CORES, NB, C, HW)
    xs = x.reshape(N_CORES, NB, C, HW)

    nc1 = _get("l1", _build_l1)
    maps = [{"xc": np.ascontiguousarray(xcs[i]), "s1": s1, "w1": w1, "w2": w2}
            for i in range(N_CORES)]
    _r = run_bass_kernel_spmd(nc1, maps, cores)
    last_exec_times.append(_r.exec_time_ns)
    r1 = _r.results

    # merge bn2 stats on host
    st = sum(np.asarray(r["st1"], np.float64).sum(axis=2) for r in r1)  # [4,128,2]
    n = N_CORES * NB * HW
    mean = st[..., 0] / n
    var = st[..., 1] / n - mean ** 2
    sc2 = np.asarray(g2).reshape(4, 128) / np.sqrt(var + EPS)
    sh2 = np.asarray(b2).reshape(4, 128) - mean * sc2
    s2 = np.stack([sc2, sh2], -1).astype(np.float32)  # [4,128,2]

    nc2 = _get("l2", _build_l2)
    maps = [{"h2": r1[i]["h2"], "s2": s2, "w3": w3} for i in range(N_CORES)]
    _r = run_bass_kernel_spmd(nc2, maps, cores)
    last_exec_times.append(_r.exec_time_ns)
    r2 = _r.results

    st = sum(np.asarray(r["st2"], np.float64).sum(axis=2) for r in r2)
    mean = st[..., 0] / n
    var = st[..., 1] / n - mean ** 2
    sc3 = np.asarray(g3).reshape(4, 128) / np.sqrt(var + EPS)
    sh3 = np.asarray(b3).reshape(4, 128) - mean * sc3
    s3 = np.stack([sc3, sh3], -1).astype(np.float32)

    nc3 = _get("l3", _build_l3)
    maps = [{"h3": r2[i]["h3"], "s3": s3, "w4": w4,
             "bo": np.asarray(b_out, np.float32).reshape(C, 1),
             "cwT": cwT,
             "cb": np.asarray(conv_b, np.float32).reshape(C, 1),
             "x": np.ascontiguousarray(xs[i])} for i in range(N_CORES)]
    _r = run_bass_kernel_spmd(nc3, maps, cores)
    last_exec_times.append(_r.exec_time_ns)
    r3 = _r.results

    z = np.concatenate(
        [np.asarray(r["z"], np.float32).reshape(NB, C, H, W) for r in r3], 0)
    ldj = []
    for r in r3:
        wt = np.asarray(r["wt"], np.float32).reshape(C, NB, C)
        for b in range(NB):
            Wb = wt[:, b, :].T.astype(np.float64)
            ldj.append(np.linalg.slogdet(Wb)[1] * HW)
    sldj_out = (np.asarray(sldj, np.float32) + np.asarray(ldj, np.float32))
    return z, sldj_out
